# revision 14
# baseline (speedup 1.0000x reference)
"""Trainium2 Bass kernel for nn_CleanAttention (sliding-window GQA attention).

Problem: x[2,4096,2048] -> qkv proj -> rope -> sliding-window (256) attention
(16 q heads, 4 kv heads, d=128) -> o proj.

Sharding: 8 cores = batch(2) x token-quarters(4). Each core computes all 16
heads for its 1024 tokens, using a 256-token key/value halo on the left.
Outputs concatenate: no inter-core reduction.

v2 design (bf16 matmul operands, fp32 accumulation/softmax):
  - x, wq, wk, wv, wo host-cast to bf16; PE runs 1 cycle/row either way but
    SBUF/DMA halve, letting all of qt (4 groups) and yt (32 chunks) stay
    resident so wq and wo stream from HBM exactly once.
  - No mask matmuls. Whole-block invalidation (first two global chunks) goes
    through the exp's per-partition bias operand (a [128,8,2] table); the
    two triangular window masks are constant 0/1 bf16 planes multiplied
    into E post-exp on the DVE.
  - xt stored chunk-major [128p, 10tb, 16kc, 128t] so each 512KB token-chunk
    DMA is contiguous and V/K projections start as soon as chunks land.
  - RoPE on fp32 psum: DVE does the cos-half + final add/sub (writing bf16),
    Pool (idle otherwise) does the sin-half multiplies.

Per-core phases (repeat-body):
  A: stream xt chunks; V proj per chunk -> v[10] bf16; K proj (3 slices/g)
     + rope -> kt[4] [128,1280] bf16
  B: Q proj per (g,h,half) + rope -> qt[4] [128,4,1024] bf16
  C: per (g,c): 3 single matmuls K^T_blk @ Q^T -> exp(scale*s + bias) -> E
     bf16 -> tri-mask muls -> sums/outt matmuls -> reciprocal -> yt bf16
  D: O proj streamed by output-column block: pos[c] += yt_m @ wo_m -> o_out
"""

import math

import ml_dtypes
import numpy as np

import concourse.bass as bass
import concourse.mybir as mybir
import concourse.tile as tile
from concourse import bacc
from concourse import bass_utils

B, T, C = 2, 4096, 2048
NH, NKV, D = 16, 4, 128
WINDOW = 256
N_CORES = 8
TCORE = 1024  # own tokens per core
HALO = 256
TX = TCORE + HALO  # 1280
NTB = TX // 128  # 10 token chunks incl. halo
NG = 4  # kv groups
GH = 4  # q heads per group
NCHUNK = 8  # query chunks of 128 per core
SCALE = 1.0 / math.sqrt(D)

f32 = mybir.dt.float32
f32r = mybir.dt.float32r
bf16 = mybir.dt.bfloat16
BF = ml_dtypes.bfloat16

_CACHE = {}


def _build_nc(repeat=1, st_bufs=4, sm_bufs=2, po_bufs=2, et_bufs=8, raw_bufs=3,
              rt_bufs=4, wq_bufs=3, wo_bufs=16, rec_bufs=3, osb_bufs=4):
    nc = bacc.Bacc("TRN2", target_bir_lowering=False, debug=False)

    xt = nc.dram_tensor("xt", [128, NTB, 16, 128], bf16, kind="ExternalInput")
    wq_t = nc.dram_tensor("wq_t", [NH, 128, 16, 128], bf16, kind="ExternalInput")
    wk_t = nc.dram_tensor("wk_t", [NKV, 128, 16, 128], bf16, kind="ExternalInput")
    wv_t = nc.dram_tensor("wv_t", [128, 16, 512], bf16, kind="ExternalInput")
    wo_t = nc.dram_tensor("wo_t", [16, 4, 128, 512], bf16, kind="ExternalInput")
    cs_t = nc.dram_tensor("cs_t", [2, 128, TX], f32, kind="ExternalInput")
    tri_t = nc.dram_tensor("tri_t", [2, 128, 512], bf16, kind="ExternalInput")
    bias_t = nc.dram_tensor("bias_t", [128, NCHUNK, 2], f32, kind="ExternalInput")
    ones_in = nc.dram_tensor("ones_in", [128, 128], bf16, kind="ExternalInput")
    o_out = nc.dram_tensor("o_out", [TCORE, C], f32, kind="ExternalOutput")

    exp_t = mybir.ActivationFunctionType.Exp

    with tile.TileContext(nc) as tc:
        with (
            tc.sbuf_pool(name="fixed", bufs=1) as fixed,
            tc.sbuf_pool(name="ktp", bufs=1) as ktp,
            tc.sbuf_pool(name="vp", bufs=1) as vp,
            tc.sbuf_pool(name="qtp", bufs=1) as qtp,
            tc.sbuf_pool(name="ytp", bufs=1) as ytp,
            tc.psum_pool(name="psA", bufs=st_bufs) as psA,
            tc.psum_pool(name="psB", bufs=sm_bufs) as psB,
            tc.psum_pool(name="psC", bufs=po_bufs) as psC,
        ):
            # constants + weights reused across reps, loaded once.
            # wv/wk first: the first rep's V/K projections wait on them.
            wv_sb = fixed.tile([128, 16, 512], bf16)
            nc.sync.dma_start(wv_sb[:], wv_t[:])
            wk_sbs = []
            for g in range(NG):
                wk_sb = fixed.tile([128, 16, 128], bf16, name=f"wkg{g}")
                nc.sync.dma_start(wk_sb[:], wk_t[g])
                wk_sbs.append(wk_sb)
            cos_sb = fixed.tile([128, TX], f32)
            nc.sync.dma_start(cos_sb[:], cs_t[0])
            sin_sb = fixed.tile([128, TX], f32)
            nc.sync.dma_start(sin_sb[:], cs_t[1])
            ones_sb = fixed.tile([128, 128], bf16)
            nc.sync.dma_start(ones_sb[:], ones_in[:])
            triu_sb = fixed.tile([128, 512], bf16)
            nc.sync.dma_start(triu_sb[:], tri_t[0])
            tril_sb = fixed.tile([128, 512], bf16)
            nc.sync.dma_start(tril_sb[:], tri_t[1])
            bias_sb = fixed.tile([128, NCHUNK, 2], f32)
            nc.sync.dma_start(bias_sb[:], bias_t[:])

            def rope(ropetmp, dst_top, dst_bot, src_psum, col0, width):
                # Evacuate psum with one ACT copy, then rotate from SBUF.
                # src rows [0:64] = even dims, [64:128] = odd dims.
                # out_top = e*cos - o*sin ; out_bot = e*sin + o*cos
                raw = ropetmp.tile([128, 512], f32, name="raw", tag="raw",
                                   bufs=raw_bufs)
                nc.scalar.copy(raw[:, :width], src_psum[:])
                sl = slice(col0, col0 + width)
                t1 = ropetmp.tile([64, 512], f32, name="t1", tag="rt")
                t2 = ropetmp.tile([64, 512], f32, name="t2", tag="rt")
                nc.vector.tensor_mul(t1[:, :width], raw[0:64, :width],
                                     cos_sb[0:64, sl])
                nc.vector.tensor_mul(t2[:, :width], raw[64:128, :width],
                                     sin_sb[64:128, sl])
                nc.vector.tensor_sub(dst_top, t1[:, :width], t2[:, :width])
                t3 = ropetmp.tile([64, 512], f32, name="t3", tag="rt")
                t4 = ropetmp.tile([64, 512], f32, name="t4", tag="rt")
                nc.gpsimd.tensor_mul(t3[:, :width], raw[0:64, :width],
                                     sin_sb[0:64, sl])
                nc.gpsimd.tensor_mul(t4[:, :width], raw[64:128, :width],
                                     cos_sb[64:128, sl])
                nc.vector.tensor_add(dst_bot, t3[:, :width], t4[:, :width])

            for rep in range(repeat):
                kt_tiles = [
                    ktp.tile([128, TX], bf16, name=f"ktg{g}", tag=f"ktg{g}")
                    for g in range(NG)
                ]
                v_tiles = [
                    vp.tile([128, 512], bf16, name=f"vtb{tb}", tag=f"vtb{tb}")
                    for tb in range(NTB)
                ]
                qt_tiles = [
                    qtp.tile([128, GH, TCORE], bf16, name=f"qtg{g}", tag=f"qtg{g}")
                    for g in range(NG)
                ]
                yts = {}
                for g in range(NG):
                    for c in range(NCHUNK):
                        yts[(g, c)] = ytp.tile(
                            [128, 512], bf16, name=f"yt{g}_{c}", tag=f"yt{g}_{c}"
                        )

                with (
                    tc.sbuf_pool(name="xtf", bufs=1) as xtf,
                    tc.sbuf_pool(name="wqp", bufs=wq_bufs) as wqp,
                    tc.sbuf_pool(name="ropetmp", bufs=rt_bufs) as ropetmp,
                ):
                    # ---- phase A: stream xt; V proj per chunk; K proj ----
                    xt_sb = xtf.tile([128, NTB, 16, 128], bf16)
                    for tb in range(NTB):
                        nc.sync.dma_start(xt_sb[:, tb], xt[:, tb])

                    for tb in range(NTB):
                        pv = psA.tile([128, 512], f32, name=f"pv{tb}", tag="st")
                        for kc in range(16):
                            nc.tensor.matmul(
                                pv[:],
                                xt_sb[:, tb, kc, :],
                                wv_sb[:, kc, :],
                                start=(kc == 0),
                                stop=(kc == 15),
                            )
                        nc.scalar.copy(v_tiles[tb][:], pv[:])

                    for g in range(NG):
                        for ts, tw in [(0, 512), (512, 512), (1024, 256)]:
                            pk = psA.tile([128, 512], f32, name=f"pk{g}_{ts}",
                                          tag="st")
                            for kc in range(16):
                                nc.tensor.matmul(
                                    pk[:, :tw],
                                    wk_sbs[g][:, kc, :],
                                    xt_sb[:, ts // 128 : (ts + tw) // 128, kc, :],
                                    start=(kc == 0),
                                    stop=(kc == 15),
                                )
                            rope(
                                ropetmp,
                                kt_tiles[g][0:64, ts : ts + tw],
                                kt_tiles[g][64:128, ts : ts + tw],
                                pk[:, :tw],
                                ts,
                                tw,
                            )

                    # ---- phase B: Q proj + rope -> qt (bf16) ----
                    for g in range(NG):
                        for mh in range(GH):
                            h = g * GH + mh
                            wq_sb = wqp.tile([128, 16, 128], bf16,
                                             name=f"wqh{h}", tag="wq")
                            nc.sync.dma_start(wq_sb[:], wq_t[h])
                            for hs in range(2):
                                tb0 = 2 + hs * 4
                                pq = psA.tile([128, 512], f32,
                                              name=f"pq{h}_{hs}", tag="st")
                                for kc in range(16):
                                    nc.tensor.matmul(
                                        pq[:],
                                        wq_sb[:, kc, :],
                                        xt_sb[:, tb0 : tb0 + 4, kc, :],
                                        start=(kc == 0),
                                        stop=(kc == 15),
                                    )
                                rope(
                                    ropetmp,
                                    qt_tiles[g][0:64, mh, hs * 512 : hs * 512 + 512],
                                    qt_tiles[g][64:128, mh, hs * 512 : hs * 512 + 512],
                                    pq[:],
                                    256 + hs * 512,
                                    512,
                                )

                # ---- phase C: attention ----
                with tc.sbuf_pool(name="etp", bufs=1) as etp:
                    for g in range(NG):
                        for c in range(NCHUNK):
                            qs = qt_tiles[g][:, :, c * 128 : c * 128 + 128]
                            sts = []
                            for kb in range(3):
                                st = psA.tile([128, 512], f32,
                                              name=f"st{g}_{c}_{kb}", tag="st")
                                nc.tensor.matmul(
                                    st[:],
                                    kt_tiles[g][:, (c + kb) * 128 : (c + kb + 1) * 128],
                                    qs,
                                    start=True,
                                    stop=True,
                                )
                                sts.append(st)
                            ets = []
                            for kb in range(3):
                                et = etp.tile([128, 512], bf16,
                                              name=f"et{g}_{c}_{kb}", tag="et",
                                              bufs=et_bufs)
                                bias = (
                                    bias_sb[:, c, 0:1] if kb == 0
                                    else bias_sb[:, c, 1:2] if kb == 1
                                    else 0.0
                                )
                                nc.scalar.activation(
                                    et[:], sts[kb][:], exp_t, bias=bias,
                                    scale=SCALE
                                )
                                ets.append(et)
                            nc.vector.tensor_mul(ets[0][:], ets[0][:], triu_sb[:])
                            nc.vector.tensor_mul(ets[2][:], ets[2][:], tril_sb[:])
                            sums = psB.tile([128, 512], f32, name=f"sm{g}_{c}",
                                            tag="sm")
                            for kb in range(3):
                                nc.tensor.matmul(
                                    sums[:], ones_sb[:], ets[kb][:],
                                    start=(kb == 0), stop=(kb == 2),
                                )
                            outt = psB.tile([128, 512], f32, name=f"ot{g}_{c}",
                                            tag="sm")
                            for kb in range(3):
                                nc.tensor.matmul(
                                    outt[:],
                                    v_tiles[c + kb][:, g * 128 : (g + 1) * 128],
                                    ets[kb][:],
                                    start=(kb == 0), stop=(kb == 2),
                                )
                            rec = etp.tile([128, 512], f32, name=f"rc{g}_{c}",
                                           tag="rec", bufs=rec_bufs)
                            nc.vector.reciprocal(rec[:], sums[:])
                            nc.vector.tensor_mul(yts[(g, c)][:], outt[:], rec[:])

                # ---- phase D: O projection (wo streamed once) ----
                with tc.sbuf_pool(name="wop", bufs=1) as wop:
                    for csx in range(4):
                        wo_sbs = []
                        for mp in range(8):
                            wo_sb = wop.tile([128, 2, 512], bf16,
                                             name=f"wo{csx}_{mp}", tag="wo",
                                             bufs=wo_bufs)
                            nc.sync.dma_start(
                                wo_sb[:],
                                wo_t[2 * mp : 2 * mp + 2, csx].rearrange(
                                    "m p n -> p m n"
                                ),
                            )
                            wo_sbs.append(wo_sb)
                        for c in range(NCHUNK):
                            pos = psC.tile([128, 512], f32, name=f"po{csx}_{c}",
                                           tag="po")
                            for mp in range(8):
                                for mi in range(2):
                                    m = 2 * mp + mi
                                    nc.tensor.matmul(
                                        pos[:],
                                        yts[(m // 4, c)][
                                            :, (m % 4) * 128 : (m % 4) * 128 + 128
                                        ],
                                        wo_sbs[mp][:, mi, :],
                                        start=(m == 0),
                                        stop=(m == 15),
                                    )
                            osb = wop.tile([128, 512], f32, name=f"osb{csx}_{c}",
                                           tag="osb", bufs=osb_bufs)
                            nc.scalar.copy(osb[:], pos[:])
                            nc.sync.dma_start(
                                o_out[
                                    c * 128 : c * 128 + 128,
                                    csx * 512 : csx * 512 + 512,
                                ],
                                osb[:],
                            )

    nc.compile()
    return nc


def _prep_shared(wq, wk, wv, wo, rope_cache):
    """Host-side weight swizzles shared by all cores."""
    perm = np.concatenate([np.arange(0, 128, 2), np.arange(1, 128, 2)])

    wq_p = wq.reshape(NH, 128, C)[:, perm, :]  # [h, d, C]
    wq_sw = np.ascontiguousarray(
        wq_p.reshape(NH, 128, 16, 128).transpose(0, 3, 2, 1)
    ).astype(BF)  # [h, p, kc, n]

    wk_p = wk.reshape(NKV, 128, C)[:, perm, :]
    wk_sw = np.ascontiguousarray(
        wk_p.reshape(NKV, 128, 16, 128).transpose(0, 3, 2, 1)
    ).astype(BF)

    wv_sw = np.ascontiguousarray(
        wv.reshape(NKV * D, 16, 128).transpose(2, 1, 0)
    ).astype(BF)  # [p, kc, n=512]

    # wo given [C, HD]; need woT tiles [m, cs, p(d), n(c)]
    wo_sw = np.ascontiguousarray(
        wo.T.reshape(16, 128, 4, 512).transpose(0, 2, 1, 3)
    ).astype(BF)

    ones = np.ones((128, 128), dtype=BF)

    # 0/1 triangle planes for key j (partition), query i, tiled over 4 heads
    j = np.arange(128)[:, None]
    i = np.arange(128)[None, :]
    triu01 = np.tile((j > i).astype(np.float32), (1, 4)).astype(BF)
    tril01 = np.tile((j <= i).astype(np.float32), (1, 4)).astype(BF)
    tri = np.stack([triu01, tril01])  # [2, 128, 512]

    return wq_sw, wk_sw, wv_sw, wo_sw, ones, tri


def _make_in_maps(x, wq, wk, wv, wo, rope_cache):
    wq_sw, wk_sw, wv_sw, wo_sw, ones, tri = _prep_shared(wq, wk, wv, wo,
                                                         rope_cache)

    in_maps = []
    for core in range(N_CORES):
        b, tq = divmod(core, 4)
        t0 = tq * TCORE

        # x^T with left halo, zero-padded below t=0, chunk-major
        xpad = np.zeros((C, TX), dtype=np.float32)
        lo = t0 - HALO
        src_lo = max(lo, 0)
        xpad[:, src_lo - lo :] = x[b, src_lo : t0 + TCORE, :].T
        # [C, TX] -> [16kc, 128p, 10tb, 128t] -> [p, tb, kc, t]
        xt_sw = np.ascontiguousarray(
            xpad.reshape(16, 128, NTB, 128).transpose(1, 2, 0, 3)
        ).astype(BF)

        # cos/sin tiles [2, 128, TX], each duplicated on both partition halves
        tglob = np.clip(np.arange(lo, t0 + TCORE), 0, T - 1)
        cs = np.empty((2, 128, TX), dtype=np.float32)
        cs[0, 0:64] = rope_cache[tglob, :, 0].T
        cs[0, 64:128] = cs[0, 0:64]
        cs[1, 0:64] = rope_cache[tglob, :, 1].T
        cs[1, 64:128] = cs[1, 0:64]

        # per-chunk whole-block exp biases: [128, c, 0]=kb0, [128, c, 1]=kb1
        bias = np.zeros((128, NCHUNK, 2), dtype=np.float32)
        for c in range(NCHUNK):
            gc = t0 // 128 + c
            if gc < 2:
                bias[:, c, 0] = -1e30
            if gc < 1:
                bias[:, c, 1] = -1e30

        in_maps.append(
            {
                "xt": xt_sw,
                "wq_t": wq_sw,
                "wk_t": wk_sw,
                "wv_t": wv_sw,
                "wo_t": wo_sw,
                "cs_t": cs,
                "tri_t": tri,
                "bias_t": bias,
                "ones_in": ones,
            }
        )
    return in_maps


def kernel(x, wq, wk, wv, wo, rope_cache):
    x = np.asarray(x, dtype=np.float32)
    wq = np.asarray(wq, dtype=np.float32)
    wk = np.asarray(wk, dtype=np.float32)
    wv = np.asarray(wv, dtype=np.float32)
    wo = np.asarray(wo, dtype=np.float32)
    rope_cache = np.asarray(rope_cache, dtype=np.float32)

    if "nc" not in _CACHE:
        _CACHE["nc"] = _build_nc()
    nc = _CACHE["nc"]

    in_maps = _make_in_maps(x, wq, wk, wv, wo, rope_cache)
    _CACHE["in_maps"] = in_maps

    res = bass_utils.run_bass_kernel_spmd(nc, in_maps, core_ids=list(range(N_CORES)))

    out = np.empty((B, T, C), dtype=np.float32)
    for core in range(N_CORES):
        b, tq = divmod(core, 4)
        out[b, tq * TCORE : (tq + 1) * TCORE, :] = res.results[core]["o_out"]
    return out


# revision 16
# speedup vs baseline: 1.0138x; 1.0138x over previous
"""Trainium2 Bass kernel for nn_CleanAttention (sliding-window GQA attention).

Problem: x[2,4096,2048] -> qkv proj -> rope -> sliding-window (256) attention
(16 q heads, 4 kv heads, d=128) -> o proj.

Sharding: 8 cores = batch(2) x token-quarters(4). Each core computes all 16
heads for its 1024 tokens, using a 256-token key/value halo on the left.
Outputs concatenate: no inter-core reduction.

v2 design (bf16 matmul operands, fp32 accumulation/softmax):
  - x, wq, wk, wv, wo host-cast to bf16; PE runs 1 cycle/row either way but
    SBUF/DMA halve, letting all of qt (4 groups) and yt (32 chunks) stay
    resident so wq and wo stream from HBM exactly once.
  - No mask matmuls. Whole-block invalidation (first two global chunks) goes
    through the exp's per-partition bias operand (a [128,8,2] table); the
    two triangular window masks are constant 0/1 bf16 planes multiplied
    into E post-exp on the DVE.
  - xt stored chunk-major [128p, 10tb, 16kc, 128t] so each 512KB token-chunk
    DMA is contiguous and V/K projections start as soon as chunks land.
  - RoPE on fp32 psum: DVE does the cos-half + final add/sub (writing bf16),
    Pool (idle otherwise) does the sin-half multiplies.

Per-core phases (repeat-body):
  A: stream xt chunks; V proj per chunk -> v[10] bf16; K proj (3 slices/g)
     + rope -> kt[4] [128,1280] bf16
  B: Q proj per (g,h,half) + rope -> qt[4] [128,4,1024] bf16
  C: per (g,c): 3 single matmuls K^T_blk @ Q^T -> exp(scale*s + bias) -> E
     bf16 -> tri-mask muls -> sums/outt matmuls -> reciprocal -> yt bf16
  D: O proj streamed by output-column block: pos[c] += yt_m @ wo_m -> o_out
"""

import math

import ml_dtypes
import numpy as np

import concourse.bass as bass
import concourse.mybir as mybir
import concourse.tile as tile
from concourse import bacc
from concourse import bass_utils

B, T, C = 2, 4096, 2048
NH, NKV, D = 16, 4, 128
WINDOW = 256
N_CORES = 8
TCORE = 1024  # own tokens per core
HALO = 256
TX = TCORE + HALO  # 1280
NTB = TX // 128  # 10 token chunks incl. halo
NG = 4  # kv groups
GH = 4  # q heads per group
NCHUNK = 8  # query chunks of 128 per core
SCALE = 1.0 / math.sqrt(D)

f32 = mybir.dt.float32
f32r = mybir.dt.float32r
bf16 = mybir.dt.bfloat16
BF = ml_dtypes.bfloat16

_CACHE = {}


def _build_nc(repeat=1, st_bufs=4, sm_bufs=2, po_bufs=2, et_bufs=8, raw_bufs=3,
              rt_bufs=4, wq_bufs=3, wo_bufs=16, rec_bufs=3, osb_bufs=4):
    nc = bacc.Bacc("TRN2", target_bir_lowering=False, debug=False)

    xt = nc.dram_tensor("xt", [128, NTB, 16, 128], bf16, kind="ExternalInput")
    wq_t = nc.dram_tensor("wq_t", [NH, 128, 16, 128], bf16, kind="ExternalInput")
    wk_t = nc.dram_tensor("wk_t", [NKV, 128, 16, 128], bf16, kind="ExternalInput")
    wv_t = nc.dram_tensor("wv_t", [128, 16, 512], bf16, kind="ExternalInput")
    wo_t = nc.dram_tensor("wo_t", [16, 4, 128, 512], bf16, kind="ExternalInput")
    cs_t = nc.dram_tensor("cs_t", [2, 128, TX], f32, kind="ExternalInput")
    tri_t = nc.dram_tensor("tri_t", [2, 128, 512], bf16, kind="ExternalInput")
    bias_t = nc.dram_tensor("bias_t", [128, NCHUNK, 2], f32, kind="ExternalInput")
    ones_in = nc.dram_tensor("ones_in", [128, 128], bf16, kind="ExternalInput")
    o_out = nc.dram_tensor("o_out", [TCORE, C], f32, kind="ExternalOutput")

    exp_t = mybir.ActivationFunctionType.Exp

    with tile.TileContext(nc) as tc:
        with (
            tc.sbuf_pool(name="fixed", bufs=1) as fixed,
            tc.sbuf_pool(name="ktp", bufs=1) as ktp,
            tc.sbuf_pool(name="vp", bufs=1) as vp,
            tc.sbuf_pool(name="qtp", bufs=1) as qtp,
            tc.sbuf_pool(name="ytp", bufs=1) as ytp,
            tc.psum_pool(name="psA", bufs=st_bufs) as psA,
            tc.psum_pool(name="psB", bufs=sm_bufs) as psB,
            tc.psum_pool(name="psC", bufs=po_bufs) as psC,
        ):
            # constants, loaded once
            cos_sb = fixed.tile([128, TX], f32)
            nc.sync.dma_start(cos_sb[:], cs_t[0])
            sin_sb = fixed.tile([128, TX], f32)
            nc.sync.dma_start(sin_sb[:], cs_t[1])
            ones_sb = fixed.tile([128, 128], bf16)
            nc.sync.dma_start(ones_sb[:], ones_in[:])
            triu_sb = fixed.tile([128, 512], bf16)
            nc.sync.dma_start(triu_sb[:], tri_t[0])
            tril_sb = fixed.tile([128, 512], bf16)
            nc.sync.dma_start(tril_sb[:], tri_t[1])
            bias_sb = fixed.tile([128, NCHUNK, 2], f32)
            nc.sync.dma_start(bias_sb[:], bias_t[:])

            def rope(ropetmp, dst_top, dst_bot, src_psum, col0, width):
                # Evacuate psum with one ACT copy, then rotate from SBUF.
                # src rows [0:64] = even dims, [64:128] = odd dims.
                # out_top = e*cos - o*sin ; out_bot = e*sin + o*cos
                raw = ropetmp.tile([128, 512], f32, name="raw", tag="raw",
                                   bufs=raw_bufs)
                nc.scalar.copy(raw[:, :width], src_psum[:])
                sl = slice(col0, col0 + width)
                t1 = ropetmp.tile([64, 512], f32, name="t1", tag="rt")
                t2 = ropetmp.tile([64, 512], f32, name="t2", tag="rt")
                nc.vector.tensor_mul(t1[:, :width], raw[0:64, :width],
                                     cos_sb[0:64, sl])
                nc.vector.tensor_mul(t2[:, :width], raw[64:128, :width],
                                     sin_sb[64:128, sl])
                nc.vector.tensor_sub(dst_top, t1[:, :width], t2[:, :width])
                t3 = ropetmp.tile([64, 512], f32, name="t3", tag="rt")
                t4 = ropetmp.tile([64, 512], f32, name="t4", tag="rt")
                nc.gpsimd.tensor_mul(t3[:, :width], raw[0:64, :width],
                                     sin_sb[0:64, sl])
                nc.gpsimd.tensor_mul(t4[:, :width], raw[64:128, :width],
                                     cos_sb[64:128, sl])
                nc.vector.tensor_add(dst_bot, t3[:, :width], t4[:, :width])

            for rep in range(repeat):
                kt_tiles = [
                    ktp.tile([128, TX], bf16, name=f"ktg{g}", tag=f"ktg{g}")
                    for g in range(NG)
                ]
                v_tiles = [
                    vp.tile([128, 512], bf16, name=f"vtb{tb}", tag=f"vtb{tb}")
                    for tb in range(NTB)
                ]
                qt_tiles = [
                    qtp.tile([128, GH, TCORE], bf16, name=f"qtg{g}", tag=f"qtg{g}")
                    for g in range(NG)
                ]
                yts = {}
                for g in range(NG):
                    for c in range(NCHUNK):
                        yts[(g, c)] = ytp.tile(
                            [128, 512], bf16, name=f"yt{g}_{c}", tag=f"yt{g}_{c}"
                        )

                with (
                    tc.sbuf_pool(name="xtf", bufs=1) as xtf,
                    tc.sbuf_pool(name="wvp", bufs=1) as wvp,
                    tc.sbuf_pool(name="wkp", bufs=2) as wkp,
                    tc.sbuf_pool(name="wqp", bufs=wq_bufs) as wqp,
                    tc.sbuf_pool(name="ropetmp", bufs=rt_bufs) as ropetmp,
                ):
                    # ---- phase A: stream xt; V proj per chunk; K proj ----
                    wv_sb = wvp.tile([128, 16, 512], bf16)
                    nc.sync.dma_start(wv_sb[:], wv_t[:])
                    xt_sb = xtf.tile([128, NTB, 16, 128], bf16)
                    for tb in range(NTB):
                        nc.sync.dma_start(xt_sb[:, tb], xt[:, tb])
                    wk_sbs = []
                    for g in range(NG):
                        wk_sb = wkp.tile([128, 16, 128], bf16, name=f"wkg{g}",
                                         tag="wk", bufs=2)
                        nc.sync.dma_start(wk_sb[:], wk_t[g])
                        wk_sbs.append(wk_sb)

                    for tb in range(NTB):
                        pv = psA.tile([128, 512], f32, name=f"pv{tb}", tag="st")
                        for kc in range(16):
                            nc.tensor.matmul(
                                pv[:],
                                xt_sb[:, tb, kc, :],
                                wv_sb[:, kc, :],
                                start=(kc == 0),
                                stop=(kc == 15),
                            )
                        nc.scalar.copy(v_tiles[tb][:], pv[:])

                    for g in range(NG):
                        for ts, tw in [(0, 512), (512, 512), (1024, 256)]:
                            pk = psA.tile([128, 512], f32, name=f"pk{g}_{ts}",
                                          tag="st")
                            for kc in range(16):
                                nc.tensor.matmul(
                                    pk[:, :tw],
                                    wk_sbs[g][:, kc, :],
                                    xt_sb[:, ts // 128 : (ts + tw) // 128, kc, :],
                                    start=(kc == 0),
                                    stop=(kc == 15),
                                )
                            rope(
                                ropetmp,
                                kt_tiles[g][0:64, ts : ts + tw],
                                kt_tiles[g][64:128, ts : ts + tw],
                                pk[:, :tw],
                                ts,
                                tw,
                            )

                    # ---- phase B: Q proj + rope -> qt (bf16) ----
                    for g in range(NG):
                        for mh in range(GH):
                            h = g * GH + mh
                            wq_sb = wqp.tile([128, 16, 128], bf16,
                                             name=f"wqh{h}", tag="wq")
                            nc.sync.dma_start(wq_sb[:], wq_t[h])
                            for hs in range(2):
                                tb0 = 2 + hs * 4
                                pq = psA.tile([128, 512], f32,
                                              name=f"pq{h}_{hs}", tag="st")
                                for kc in range(16):
                                    nc.tensor.matmul(
                                        pq[:],
                                        wq_sb[:, kc, :],
                                        xt_sb[:, tb0 : tb0 + 4, kc, :],
                                        start=(kc == 0),
                                        stop=(kc == 15),
                                    )
                                rope(
                                    ropetmp,
                                    qt_tiles[g][0:64, mh, hs * 512 : hs * 512 + 512],
                                    qt_tiles[g][64:128, mh, hs * 512 : hs * 512 + 512],
                                    pq[:],
                                    256 + hs * 512,
                                    512,
                                )

                # ---- phase C: attention ----
                with tc.sbuf_pool(name="etp", bufs=1) as etp:
                    for g in range(NG):
                        for c in range(NCHUNK):
                            qs = qt_tiles[g][:, :, c * 128 : c * 128 + 128]
                            sts = []
                            for kb in range(3):
                                st = psA.tile([128, 512], f32,
                                              name=f"st{g}_{c}_{kb}", tag="st")
                                nc.tensor.matmul(
                                    st[:],
                                    kt_tiles[g][:, (c + kb) * 128 : (c + kb + 1) * 128],
                                    qs,
                                    start=True,
                                    stop=True,
                                )
                                sts.append(st)
                            ets = []
                            for kb in range(3):
                                et = etp.tile([128, 512], bf16,
                                              name=f"et{g}_{c}_{kb}", tag="et",
                                              bufs=et_bufs)
                                bias = (
                                    bias_sb[:, c, 0:1] if kb == 0
                                    else bias_sb[:, c, 1:2] if kb == 1
                                    else 0.0
                                )
                                nc.scalar.activation(
                                    et[:], sts[kb][:], exp_t, bias=bias,
                                    scale=SCALE
                                )
                                ets.append(et)
                            nc.vector.tensor_mul(ets[0][:], ets[0][:], triu_sb[:])
                            nc.vector.tensor_mul(ets[2][:], ets[2][:], tril_sb[:])
                            sums = psB.tile([128, 512], f32, name=f"sm{g}_{c}",
                                            tag="sm")
                            for kb in range(3):
                                nc.tensor.matmul(
                                    sums[:], ones_sb[:], ets[kb][:],
                                    start=(kb == 0), stop=(kb == 2),
                                )
                            outt = psB.tile([128, 512], f32, name=f"ot{g}_{c}",
                                            tag="sm")
                            for kb in range(3):
                                nc.tensor.matmul(
                                    outt[:],
                                    v_tiles[c + kb][:, g * 128 : (g + 1) * 128],
                                    ets[kb][:],
                                    start=(kb == 0), stop=(kb == 2),
                                )
                            rec = etp.tile([128, 512], f32, name=f"rc{g}_{c}",
                                           tag="rec", bufs=rec_bufs)
                            nc.vector.reciprocal(rec[:], sums[:])
                            nc.vector.tensor_mul(yts[(g, c)][:], outt[:], rec[:])

                # ---- phase D: O projection (wo streamed once) ----
                with tc.sbuf_pool(name="wop", bufs=1) as wop:
                    for csx in range(4):
                        wo_sbs = []
                        for mp in range(8):
                            wo_sb = wop.tile([128, 2, 512], bf16,
                                             name=f"wo{csx}_{mp}", tag="wo",
                                             bufs=wo_bufs)
                            nc.sync.dma_start(
                                wo_sb[:],
                                wo_t[2 * mp : 2 * mp + 2, csx].rearrange(
                                    "m p n -> p m n"
                                ),
                            )
                            wo_sbs.append(wo_sb)
                        for c in range(NCHUNK):
                            pos = psC.tile([128, 512], f32, name=f"po{csx}_{c}",
                                           tag="po")
                            for mp in range(8):
                                for mi in range(2):
                                    m = 2 * mp + mi
                                    nc.tensor.matmul(
                                        pos[:],
                                        yts[(m // 4, c)][
                                            :, (m % 4) * 128 : (m % 4) * 128 + 128
                                        ],
                                        wo_sbs[mp][:, mi, :],
                                        start=(m == 0),
                                        stop=(m == 15),
                                    )
                            osb = wop.tile([128, 512], f32, name=f"osb{csx}_{c}",
                                           tag="osb", bufs=osb_bufs)
                            nc.scalar.copy(osb[:], pos[:])
                            nc.sync.dma_start(
                                o_out[
                                    c * 128 : c * 128 + 128,
                                    csx * 512 : csx * 512 + 512,
                                ],
                                osb[:],
                            )

    nc.compile()
    return nc


def _prep_shared(wq, wk, wv, wo, rope_cache):
    """Host-side weight swizzles shared by all cores."""
    perm = np.concatenate([np.arange(0, 128, 2), np.arange(1, 128, 2)])

    wq_p = wq.reshape(NH, 128, C)[:, perm, :]  # [h, d, C]
    wq_sw = np.ascontiguousarray(
        wq_p.reshape(NH, 128, 16, 128).transpose(0, 3, 2, 1)
    ).astype(BF)  # [h, p, kc, n]

    wk_p = wk.reshape(NKV, 128, C)[:, perm, :]
    wk_sw = np.ascontiguousarray(
        wk_p.reshape(NKV, 128, 16, 128).transpose(0, 3, 2, 1)
    ).astype(BF)

    wv_sw = np.ascontiguousarray(
        wv.reshape(NKV * D, 16, 128).transpose(2, 1, 0)
    ).astype(BF)  # [p, kc, n=512]

    # wo given [C, HD]; need woT tiles [m, cs, p(d), n(c)]
    wo_sw = np.ascontiguousarray(
        wo.T.reshape(16, 128, 4, 512).transpose(0, 2, 1, 3)
    ).astype(BF)

    ones = np.ones((128, 128), dtype=BF)

    # 0/1 triangle planes for key j (partition), query i, tiled over 4 heads
    j = np.arange(128)[:, None]
    i = np.arange(128)[None, :]
    triu01 = np.tile((j > i).astype(np.float32), (1, 4)).astype(BF)
    tril01 = np.tile((j <= i).astype(np.float32), (1, 4)).astype(BF)
    tri = np.stack([triu01, tril01])  # [2, 128, 512]

    return wq_sw, wk_sw, wv_sw, wo_sw, ones, tri


def _make_in_maps(x, wq, wk, wv, wo, rope_cache):
    wq_sw, wk_sw, wv_sw, wo_sw, ones, tri = _prep_shared(wq, wk, wv, wo,
                                                         rope_cache)

    in_maps = []
    for core in range(N_CORES):
        b, tq = divmod(core, 4)
        t0 = tq * TCORE

        # x^T with left halo, zero-padded below t=0, chunk-major
        xpad = np.zeros((C, TX), dtype=np.float32)
        lo = t0 - HALO
        src_lo = max(lo, 0)
        xpad[:, src_lo - lo :] = x[b, src_lo : t0 + TCORE, :].T
        # [C, TX] -> [16kc, 128p, 10tb, 128t] -> [p, tb, kc, t]
        xt_sw = np.ascontiguousarray(
            xpad.reshape(16, 128, NTB, 128).transpose(1, 2, 0, 3)
        ).astype(BF)

        # cos/sin tiles [2, 128, TX], each duplicated on both partition halves
        tglob = np.clip(np.arange(lo, t0 + TCORE), 0, T - 1)
        cs = np.empty((2, 128, TX), dtype=np.float32)
        cs[0, 0:64] = rope_cache[tglob, :, 0].T
        cs[0, 64:128] = cs[0, 0:64]
        cs[1, 0:64] = rope_cache[tglob, :, 1].T
        cs[1, 64:128] = cs[1, 0:64]

        # per-chunk whole-block exp biases: [128, c, 0]=kb0, [128, c, 1]=kb1
        bias = np.zeros((128, NCHUNK, 2), dtype=np.float32)
        for c in range(NCHUNK):
            gc = t0 // 128 + c
            if gc < 2:
                bias[:, c, 0] = -1e30
            if gc < 1:
                bias[:, c, 1] = -1e30

        in_maps.append(
            {
                "xt": xt_sw,
                "wq_t": wq_sw,
                "wk_t": wk_sw,
                "wv_t": wv_sw,
                "wo_t": wo_sw,
                "cs_t": cs,
                "tri_t": tri,
                "bias_t": bias,
                "ones_in": ones,
            }
        )
    return in_maps


def kernel(x, wq, wk, wv, wo, rope_cache):
    x = np.asarray(x, dtype=np.float32)
    wq = np.asarray(wq, dtype=np.float32)
    wk = np.asarray(wk, dtype=np.float32)
    wv = np.asarray(wv, dtype=np.float32)
    wo = np.asarray(wo, dtype=np.float32)
    rope_cache = np.asarray(rope_cache, dtype=np.float32)

    if "nc" not in _CACHE:
        _CACHE["nc"] = _build_nc()
    nc = _CACHE["nc"]

    in_maps = _make_in_maps(x, wq, wk, wv, wo, rope_cache)
    _CACHE["in_maps"] = in_maps

    res = bass_utils.run_bass_kernel_spmd(nc, in_maps, core_ids=list(range(N_CORES)))

    out = np.empty((B, T, C), dtype=np.float32)
    for core in range(N_CORES):
        b, tq = divmod(core, 4)
        out[b, tq * TCORE : (tq + 1) * TCORE, :] = res.results[core]["o_out"]
    return out
